# revision 28
# baseline (speedup 1.0000x reference)
"""Self-attention kernel for TRN2, data-parallel over batch (8 cores), fp8.

Per core (one batch element x[2048, 512]):
  - x loaded fp32 (residual), cast bf16 on ScalarE, transposed on TensorE
    to xT (c on partitions), stored fp8.
  - q/k/v projections and all attention matmuls run fp8 with
    perf_mode=DoubleRow (contraction pairs of 128-chunks -> ~2x TensorE).
  - scores computed TRANSPOSED (sT[s,t]) so the exp output feeds PV
    directly; exp = e^{score/16 - 2} (bias cancels in normalization),
    fused over two PSUM banks per activation.
  - PV runs all four 128-row output accumulators lag-1 behind the exp so
    there is no drain tail; row sums come free via a ones-column in v;
    the reciprocal is folded into the bf16 cast of a (per-partition
    scalar), so the output projection needs no further scaling.
  - qT for block tb+1, transposes/output-projection of block tb-1, and
    the residual-bias adds are interleaved into block tb's score streak.
  - biases: bq/bk exact via per-partition add; bv/ba folded on the HOST
    into bc = Wa^T bv + ba, added into the fp32 residual x (exact:
    attention rows sum to 1).

Matmul inputs fp8e4, PSUM accumulation fp32, softmax/normalize/residual fp32.
"""

import ml_dtypes
import numpy as np

import concourse.bass as bass
import concourse.mybir as mybir
import concourse.tile as tile
from concourse import bacc
from concourse.bass import ds, ts
from concourse.bass_utils import run_bass_kernel_spmd
from concourse.masks import make_identity

F32 = mybir.dt.float32
BF16 = mybir.dt.bfloat16
F8 = mybir.dt.float8e4
AF = mybir.ActivationFunctionType
DR = mybir.MatmulPerfMode.DoubleRow

B, T, C, U, P = 8, 2048, 512, 256, 128
TC = T // P    # 16 row tiles
CCH = C // P   # 4 c-chunks
UCH = U // P   # 2 u-chunks
TBLK = 512     # t-block for attention
NTB = T // TBLK  # 4
VF = U + 16    # v free dim padded so the pair-dim stride is 16B-aligned
SCALE = 1.0 / float(np.sqrt(U))
EXPB = -2.0    # exp bias; cancels in row-sum normalization

_cache = {}


def _build_kernel(tc):
    nc = tc.nc
    x = nc.dram_tensor("x", [T, C], F32, kind="ExternalInput").ap()
    xbf = nc.dram_tensor("xbf", [T, C], BF16, kind="ExternalInput").ap()
    Wq8 = nc.dram_tensor("Wq8", [P, CCH, U], F8, kind="ExternalInput").ap()
    Wk8 = nc.dram_tensor("Wk8", [P, CCH, U], F8, kind="ExternalInput").ap()
    Wv8 = nc.dram_tensor("Wv8", [P, CCH, U], F8, kind="ExternalInput").ap()
    Wa8 = nc.dram_tensor("Wa8", [P, UCH, C], F8, kind="ExternalInput").ap()
    bq = nc.dram_tensor("bq", [U], F32, kind="ExternalInput").ap()
    bk = nc.dram_tensor("bk", [U], F32, kind="ExternalInput").ap()
    bcrep = nc.dram_tensor("bcrep", [P, C], F32, kind="ExternalInput").ap()
    out = nc.dram_tensor("out", [T, C], F32, kind="ExternalOutput").ap()

    consts = tc.alloc_tile_pool(name="consts", bufs=1)
    persist = tc.alloc_tile_pool(name="persist", bufs=1)

    identity = consts.tile([P, P], BF16)
    make_identity(nc, identity)

    # warm the ACT exp table early (one-time ~2.7us table load)
    dex = consts.tile([P, 1], F32)
    nc.vector.memset(dex, 0.0)
    expb = consts.tile([P, 1], F32)
    nc.vector.memset(expb, EXPB)
    dex2 = consts.tile([P, 1], F32)
    nc.scalar.activation(out=dex2, in_=dex, func=AF.Exp, bias=dex[:, 0:1],
                         scale=1.0)

    # persistent tensors
    x_sb = persist.tile([P, TC, C], F32)      # x rows (+bc), fp32 residual
    xbf_sb = persist.tile([P, TC, C], BF16)   # x rows bf16 (host-cast)
    xT_f8 = persist.tile([P, CCH, T], F8)     # x^T  (c on partitions)
    qT_f8 = persist.tile([P, UCH, T], F8)     # q^T  (u on partitions)
    kT_f8 = persist.tile([P, UCH, T], F8)     # k^T
    v_sb = persist.tile([P, TC, VF], F8)      # v rows + ones col + pad
    aT_f8 = persist.tile([P, UCH, T], F8)     # a^T (normalized)
    nc.vector.memset(v_sb[:, :, U:VF], 0.0)
    nc.vector.memset(v_sb[:, :, U:U + 1], 1.0)

    # input DMAs: ALL on the sync queue, in consumption order.  The
    # bf16 copy of x (host-cast) streams first and gates the transpose /
    # projection pipeline; the fp32 x is residual-only and arrives later.
    xr = x.rearrange("(tt p) c -> p tt c", p=P)
    xbr = xbf.rearrange("(tt p) c -> p tt c", p=P)
    for k in range(TC // 2):
        nc.sync.dma_start(out=xbf_sb[:, 2 * k:2 * k + 2, :],
                          in_=xbr[:, 2 * k:2 * k + 2, :])
    Wq_f8 = consts.tile([P, CCH, U], F8)
    nc.sync.dma_start(out=Wq_f8, in_=Wq8)
    Wk_f8 = consts.tile([P, CCH, U], F8)
    nc.sync.dma_start(out=Wk_f8, in_=Wk8)
    Wv_f8 = consts.tile([P, CCH, U], F8)
    nc.sync.dma_start(out=Wv_f8, in_=Wv8)
    Wa_f8 = consts.tile([P, UCH, C], F8)
    nc.sync.dma_start(out=Wa_f8, in_=Wa8)
    bq_sb = consts.tile([P, UCH], F32)
    nc.sync.dma_start(out=bq_sb, in_=bq.rearrange("(uc p) -> p uc", p=P))
    bk_sb = consts.tile([P, UCH], F32)
    nc.sync.dma_start(out=bk_sb, in_=bk.rearrange("(uc p) -> p uc", p=P))
    bc_sb = consts.tile([P, C], F32)
    nc.sync.dma_start(out=bc_sb, in_=bcrep)
    for k in range(TC // 2):
        nc.sync.dma_start(out=x_sb[:, 2 * k:2 * k + 2, :],
                          in_=xr[:, 2 * k:2 * k + 2, :])

    warm_rhs = consts.tile([P, TBLK], BF16)
    nc.vector.memset(warm_rhs, 0.0)
    # PE warmup during the x DMA (N=512: denser activity for HAM)
    with tc.tile_pool(name="warm", bufs=1, space="PSUM") as warm_pool:
        wtile = warm_pool.tile([P, TBLK], F32, name="warmup")
        for i in range(4):
            nc.tensor.matmul(wtile, lhsT=identity, rhs=warm_rhs,
                             start=(i == 0), stop=(i == 3))

    # --- phases 1+2 interleaved per t-block: transposes for a block's
    # four x tiles, then its kT (+ qT for block 0) and v projections, so
    # projection matmuls overlap the x DMA stream ---
    def proj_group(wpool, W_f8, bias_sb, dst, uc, tb, eng_act):
        wps = wpool.tile([P, TBLK], F32, tag="wps", name="wps")
        for i in range(2):
            nc.tensor.matmul(
                wps,
                lhsT=W_f8[:, 2 * i:2 * i + 2, ts(uc, P)],
                rhs=xT_f8[:, 2 * i:2 * i + 2, ds(tb * TBLK, TBLK)],
                start=(i == 0), stop=(i == 1), perf_mode=DR,
            )
        if eng_act:
            nc.scalar.activation(
                out=dst[:, uc, ds(tb * TBLK, TBLK)], in_=wps,
                func=AF.Identity, bias=bias_sb[:, uc:uc + 1], scale=1.0,
            )
        else:
            nc.vector.tensor_scalar(
                out=dst[:, uc, ds(tb * TBLK, TBLK)], in0=wps,
                scalar1=bias_sb[:, uc:uc + 1], scalar2=None,
                op0=mybir.AluOpType.add,
            )

    with tc.tile_pool(name="tpsum", bufs=3, space="PSUM") as tpsum, \
         tc.tile_pool(name="wpsum", bufs=3, space="PSUM") as wpsum, \
         tc.tile_pool(name="vpsum", bufs=2, space="PSUM") as vpsum:
        for tb in range(NTB):
            for tt in range(tb * 4, tb * 4 + 4):
                # transpose via NORMAL matmul against identity (exact for
                # bf16, pipelines fast and keeps the HAM activity
                # monitor warm); reads the host-cast bf16 x directly
                tps = tpsum.tile([P, CCH, P], F32, tag="tps")
                for cc in range(CCH):
                    nc.tensor.matmul(
                        tps[:, cc, :], lhsT=xbf_sb[:, tt, ts(cc, P)],
                        rhs=identity,
                        start=(cc == 0), stop=(cc == CCH - 1),
                    )
                if tt % 2 == 0:
                    nc.vector.tensor_copy(
                        out=xT_f8[:, :, ts(tt, P)], in_=tps
                    )
                else:
                    nc.scalar.copy(out=xT_f8[:, :, ts(tt, P)], in_=tps)
            for uc in range(UCH):
                proj_group(wpsum, Wk_f8, bk_sb, kT_f8, uc, tb, uc == 0)
            if tb == 0:
                for uc in range(UCH):
                    proj_group(wpsum, Wq_f8, bq_sb, qT_f8, uc, 0, uc == 0)
            # v for tiles of this block: two row tiles share one PSUM
            # bank (the second pair opens with start=False so the
            # bank-wide clear of the first start doesn't zero it)
            for h in range(2):
                t0 = tb * 4 + 2 * h
                vps = vpsum.tile([P, 2, U], F32, tag="vps")
                for j in range(2):
                    for i in range(2):
                        nc.tensor.matmul(
                            vps[:, j, :],
                            lhsT=xT_f8[:, 2 * i:2 * i + 2, ts(t0 + j, P)],
                            rhs=Wv_f8[:, 2 * i:2 * i + 2, :],
                            start=(j == 0 and i == 0),
                            stop=(j == 1 and i == 1), perf_mode=DR,
                        )
                if h == 0:
                    nc.scalar.copy(out=v_sb[:, t0:t0 + 2, 0:U], in_=vps)
                else:
                    nc.vector.tensor_copy(
                        out=v_sb[:, t0:t0 + 2, 0:U], in_=vps
                    )

    # --- phase 3: attention ---
    # PSUM: spsum 2x(2 banks) for scores, apsum "acc" 2x(1 bank) holding
    # two 128-row PV accumulators per bank (second half opens with
    # start=False so the bank clear of the first doesn't zero it), and
    # "misc" 2x(1 bank) for a-transposes / output projection / next-qT.
    spsum = tc.alloc_tile_pool(name="spsum", bufs=2, space="PSUM")
    apsum = tc.alloc_tile_pool(name="apsum", bufs=2, space="PSUM")
    p_pool = tc.alloc_tile_pool(name="p_pool", bufs=12)
    abf_pool = tc.alloc_tile_pool(name="abf_pool", bufs=6)
    rcp_pool = tc.alloc_tile_pool(name="rcp_pool", bufs=3)
    y_pool = tc.alloc_tile_pool(name="y_pool", bufs=2)

    def norm_cast(apss, abfs, tsl):
        """rcp of row sum, then a_bf = aps * rcp (normalized), fp32->bf16."""
        aps = apss[tsl]
        rcp = rcp_pool.tile([P, 1], F32, tag="rcp")
        nc.vector.reciprocal(rcp, aps[:, U:U + 1])
        a_bf = abf_pool.tile([P, U], BF16, tag="abf")
        nc.vector.tensor_scalar(
            out=a_bf, in0=aps[:, 0:U], scalar1=rcp, scalar2=None,
            op0=mybir.AluOpType.mult,
        )
        abfs[tsl] = a_bf

    def deferred_work(tb, abfs):
        """Transposes of a (tb), then output projection + residual (tb).
        y tiles pair up for one DMA per two row tiles."""
        chunks = []
        y2box = [None]
        for tsl in range(NTB):
            def tchunk(tsl=tsl, tb=tb, abfs=abfs):
                row0 = tb * TBLK + tsl * P
                atps = apsum.tile([P, UCH, P], F32, tag="misc", name="atps")
                for uc in range(UCH):
                    nc.tensor.matmul(
                        atps[:, uc, :], lhsT=abfs[tsl][:, ts(uc, P)],
                        rhs=identity,
                        start=(uc == 0), stop=(uc == UCH - 1),
                    )
                nc.vector.tensor_copy(out=aT_f8[:, :, ds(row0, P)], in_=atps)
            chunks.append(tchunk)
        for tsl in range(NTB):
            def fchunk(tsl=tsl, tb=tb, y2box=y2box):
                row0 = tb * TBLK + tsl * P
                yps = apsum.tile([P, TBLK], F32, tag="misc", name="yps")
                nc.tensor.matmul(
                    yps, lhsT=aT_f8[:, :, ds(row0, P)], rhs=Wa_f8[:, :, :],
                    start=True, stop=True, perf_mode=DR,
                )
                if tsl % 2 == 0:
                    y2box[0] = y_pool.tile([P, 2, C], F32, tag="ysb",
                                           name="y2")
                y2 = y2box[0]
                nc.vector.tensor_add(
                    out=y2[:, tsl % 2, :], in0=yps,
                    in1=x_sb[:, tb * NTB + tsl, :]
                )
                if tsl % 2 == 1:
                    orow = tb * TBLK + (tsl - 1) * P
                    nc.sync.dma_start(
                        out=out[ds(orow, 2 * P), :].rearrange(
                            "(j p) c -> p j c", p=P),
                        in_=y2,
                    )
            chunks.append(fchunk)
        return chunks

    def emit_scp(tb, scp, pts):
        sps = spsum.tile([P, 2, TBLK], F32, tag="sps", name="sps")
        for j in range(2):
            nc.tensor.matmul(
                sps[:, j, :],
                lhsT=kT_f8[:, :, ts(2 * scp + j, P)],
                rhs=qT_f8[:, :, ds(tb * TBLK, TBLK)],
                start=True, stop=True, perf_mode=DR,
            )
        pt = p_pool.tile([P, 2, TBLK], F8, tag="pt", name="pt")
        nc.scalar.activation(out=pt, in_=sps, func=AF.Exp,
                             bias=expb[:, 0:1], scale=SCALE)
        pts.append(pt)

    deferred = []
    nextpts = []
    for tb in range(NTB):
        pts = nextpts  # scp0 may have been hoisted into tb-1's tail
        nextpts = []
        abfs = [None] * NTB
        apss = [None] * NTB
        for tsl in (0, 1):
            apss[tsl] = apsum.tile([P, VF], F32, tag="acc", name="apsA")
        todo = list(deferred)  # deferred chunks from tb-1
        npv = [0]  # sweep-A pairs emitted so far

        def pva_upto(limit):
            while npv[0] < limit:
                j = npv[0]
                for tsl in (0, 1):
                    nc.tensor.matmul(
                        apss[tsl],
                        lhsT=pts[j][:, :, ts(tsl, P)],
                        rhs=v_sb[:, 2 * j:2 * j + 2, :],
                        start=(j == 0), stop=False, perf_mode=DR,
                    )
                npv[0] += 1

        for scp in range(len(pts), 8):
            emit_scp(tb, scp, pts)
            # PV sweep A (row tiles 0,1), one pair behind the exp
            pva_upto(scp)
            # interleave deferred transposes/output-proj of tb-1
            if scp >= 2:
                while todo and len(todo) > (7 - scp):
                    todo.pop(0)()
            # residual bias add for this block's x tiles (needed by the
            # output projection one block later)
            if scp >= 4:
                tt = tb * 4 + scp - 4
                nc.vector.tensor_add(out=x_sb[:, tt, :],
                                     in0=x_sb[:, tt, :], in1=bc_sb)
        pva_upto(7)
        for tsl in (0, 1):
            nc.tensor.matmul(
                apss[tsl], lhsT=pts[7][:, :, ts(tsl, P)],
                rhs=v_sb[:, 14:16, :], start=False, stop=True, perf_mode=DR,
            )
        while todo:
            todo.pop(0)()
        # produce qT for the NEXT block while the sweep-A drain happens
        if tb + 1 < NTB:
            for uc in range(UCH):
                wps = apsum.tile([P, TBLK], F32, tag="misc", name="qps")
                for i in range(2):
                    nc.tensor.matmul(
                        wps,
                        lhsT=Wq_f8[:, 2 * i:2 * i + 2, ts(uc, P)],
                        rhs=xT_f8[:, 2 * i:2 * i + 2,
                                  ds((tb + 1) * TBLK, TBLK)],
                        start=(i == 0), stop=(i == 1), perf_mode=DR,
                    )
                nc.scalar.activation(
                    out=qT_f8[:, uc, ds((tb + 1) * TBLK, TBLK)],
                    in_=wps,
                    func=AF.Identity, bias=bq_sb[:, uc:uc + 1], scale=1.0,
                )
            # hoist the next block's first two score groups so their
            # exps run on ScalarE while sweep B occupies the PE
            emit_scp(tb + 1, 0, nextpts)
            emit_scp(tb + 1, 1, nextpts)
        norm_cast(apss, abfs, 0)
        norm_cast(apss, abfs, 1)
        # PV sweep B (row tiles 2,3) over the retained p tiles
        for tsl in (2, 3):
            apss[tsl] = apsum.tile([P, VF], F32, tag="acc", name="apsB")
        for scp in range(8):
            for tsl in (2, 3):
                nc.tensor.matmul(
                    apss[tsl],
                    lhsT=pts[scp][:, :, ts(tsl, P)],
                    rhs=v_sb[:, 2 * scp:2 * scp + 2, :],
                    start=(scp == 0), stop=(scp == 7), perf_mode=DR,
                )
        norm_cast(apss, abfs, 2)
        norm_cast(apss, abfs, 3)
        if tb < NTB - 1:
            deferred = deferred_work(tb, abfs)
        else:
            # last block: emit immediately to shorten the tail
            for chunk in deferred_work(tb, abfs):
                chunk()

    for pool in (y_pool, rcp_pool, abf_pool, p_pool,
                 apsum, spsum, persist, consts):
        pool.release()


def _get_nc():
    if "nc" not in _cache:
        nc = bacc.Bacc("TRN2", target_bir_lowering=False, debug=False)
        with tile.TileContext(nc) as tc:
            _build_kernel(tc)
        nc.compile()
        _cache["nc"] = nc
    return _cache["nc"]


def _w8(w, chunks):
    """fp32 [K, N] -> fp8e4m3 [P, K//P, N] with K-chunk layout for lhsT."""
    f8 = w.reshape(chunks, P, -1).transpose(1, 0, 2)
    return np.ascontiguousarray(f8.astype(ml_dtypes.float8_e4m3))


def _host_inputs(inputs):
    f32 = np.float32
    Wa = np.ascontiguousarray(np.asarray(inputs["Wa"], dtype=f32))
    bc = np.asarray(inputs["bv"], dtype=f32) @ Wa + np.asarray(
        inputs["ba"], dtype=f32
    )
    bcrep = np.ascontiguousarray(
        np.broadcast_to(bc[None, :], (P, C)), dtype=f32
    )
    shared = {
        "Wq8": _w8(np.asarray(inputs["Wq"], dtype=f32), CCH),
        "Wk8": _w8(np.asarray(inputs["Wk"], dtype=f32), CCH),
        "Wv8": _w8(np.asarray(inputs["Wv"], dtype=f32), CCH),
        "Wa8": _w8(Wa, UCH),
        "bq": np.ascontiguousarray(np.asarray(inputs["bq"], dtype=f32)),
        "bk": np.ascontiguousarray(np.asarray(inputs["bk"], dtype=f32)),
        "bcrep": bcrep,
    }
    xs = np.ascontiguousarray(np.asarray(inputs["x"], dtype=f32))
    xbs = np.ascontiguousarray(xs.astype(ml_dtypes.bfloat16))
    return [dict(shared, x=xs[b], xbf=xbs[b]) for b in range(B)]


def kernel(**inputs):
    nc = _get_nc()
    in_maps = _host_inputs(inputs)
    res = run_bass_kernel_spmd(nc, in_maps, core_ids=list(range(B)))
    return np.stack([res.results[b]["out"] for b in range(B)], axis=0)


# revision 29
# speedup vs baseline: 1.0526x; 1.0526x over previous
"""Self-attention kernel for TRN2, data-parallel over batch (8 cores), fp8.

Per core (one batch element x[2048, 512]):
  - x loaded fp32 (residual), cast bf16 on ScalarE, transposed on TensorE
    to xT (c on partitions), stored fp8.
  - q/k/v projections and all attention matmuls run fp8 with
    perf_mode=DoubleRow (contraction pairs of 128-chunks -> ~2x TensorE).
  - scores computed TRANSPOSED (sT[s,t]) so the exp output feeds PV
    directly; exp = e^{score/16 - 2} (bias cancels in normalization),
    fused over two PSUM banks per activation.
  - PV runs all four 128-row output accumulators lag-1 behind the exp so
    there is no drain tail; row sums come free via a ones-column in v;
    the reciprocal is folded into the bf16 cast of a (per-partition
    scalar), so the output projection needs no further scaling.
  - qT for block tb+1, transposes/output-projection of block tb-1, and
    the residual-bias adds are interleaved into block tb's score streak.
  - biases: bq/bk exact via per-partition add; bv/ba folded on the HOST
    into bc = Wa^T bv + ba, added into the fp32 residual x (exact:
    attention rows sum to 1).

Matmul inputs fp8e4, PSUM accumulation fp32, softmax/normalize/residual fp32.
"""

import ml_dtypes
import numpy as np

import concourse.bass as bass
import concourse.mybir as mybir
import concourse.tile as tile
from concourse import bacc
from concourse.bass import ds, ts
from concourse.bass_utils import run_bass_kernel_spmd
from concourse.masks import make_identity

F32 = mybir.dt.float32
BF16 = mybir.dt.bfloat16
F8 = mybir.dt.float8e4
AF = mybir.ActivationFunctionType
DR = mybir.MatmulPerfMode.DoubleRow

B, T, C, U, P = 8, 2048, 512, 256, 128
TC = T // P    # 16 row tiles
CCH = C // P   # 4 c-chunks
UCH = U // P   # 2 u-chunks
TBLK = 512     # t-block for attention
NTB = T // TBLK  # 4
VF = U + 16    # v free dim padded so the pair-dim stride is 16B-aligned
SCALE = 1.0 / float(np.sqrt(U))
EXPB = -2.0    # exp bias; cancels in row-sum normalization

_cache = {}


def _build_kernel(tc):
    nc = tc.nc
    x = nc.dram_tensor("x", [T, C], F32, kind="ExternalInput").ap()
    xbf = nc.dram_tensor("xbf", [T, C], BF16, kind="ExternalInput").ap()
    Wq8 = nc.dram_tensor("Wq8", [P, CCH, U], F8, kind="ExternalInput").ap()
    Wk8 = nc.dram_tensor("Wk8", [P, CCH, U], F8, kind="ExternalInput").ap()
    Wv8 = nc.dram_tensor("Wv8", [P, CCH, U], F8, kind="ExternalInput").ap()
    Wa8 = nc.dram_tensor("Wa8", [P, UCH, C], F8, kind="ExternalInput").ap()
    bq = nc.dram_tensor("bq", [U], F32, kind="ExternalInput").ap()
    bk = nc.dram_tensor("bk", [U], F32, kind="ExternalInput").ap()
    bcrep = nc.dram_tensor("bcrep", [P, C], F32, kind="ExternalInput").ap()
    out = nc.dram_tensor("out", [T, C], F32, kind="ExternalOutput").ap()

    consts = tc.alloc_tile_pool(name="consts", bufs=1)
    persist = tc.alloc_tile_pool(name="persist", bufs=1)

    identity = consts.tile([P, P], BF16)
    make_identity(nc, identity)

    # warm the ACT exp table early (one-time ~2.7us table load)
    dex = consts.tile([P, 1], F32)
    nc.vector.memset(dex, 0.0)
    expb = consts.tile([P, 1], F32)
    nc.vector.memset(expb, EXPB)
    dex2 = consts.tile([P, 1], F32)
    nc.scalar.activation(out=dex2, in_=dex, func=AF.Exp, bias=dex[:, 0:1],
                         scale=1.0)

    # persistent tensors
    x_sb = persist.tile([P, TC, C], F32)      # x rows (+bc), fp32 residual
    xbf_sb = persist.tile([P, TC, C], BF16)   # x rows bf16 (host-cast)
    xT_f8 = persist.tile([P, CCH, T], F8)     # x^T  (c on partitions)
    qT_f8 = persist.tile([P, UCH, T], F8)     # q^T  (u on partitions)
    kT_f8 = persist.tile([P, UCH, T], F8)     # k^T
    v_sb = persist.tile([P, TC, VF], F8)      # v rows + ones col + pad
    aT_f8 = persist.tile([P, UCH, T], F8)     # a^T (normalized)
    nc.vector.memset(v_sb[:, :, U:VF], 0.0)
    nc.vector.memset(v_sb[:, :, U:U + 1], 1.0)

    # input DMAs: ALL on the sync queue, in consumption order.  The
    # bf16 copy of x (host-cast) streams first and gates the transpose /
    # projection pipeline; the fp32 x is residual-only and arrives later.
    xr = x.rearrange("(tt p) c -> p tt c", p=P)
    xbr = xbf.rearrange("(tt p) c -> p tt c", p=P)
    for k in range(TC // 2):
        nc.sync.dma_start(out=xbf_sb[:, 2 * k:2 * k + 2, :],
                          in_=xbr[:, 2 * k:2 * k + 2, :])
    Wq_f8 = consts.tile([P, CCH, U], F8)
    nc.sync.dma_start(out=Wq_f8, in_=Wq8)
    Wk_f8 = consts.tile([P, CCH, U], F8)
    nc.sync.dma_start(out=Wk_f8, in_=Wk8)
    Wv_f8 = consts.tile([P, CCH, U], F8)
    nc.sync.dma_start(out=Wv_f8, in_=Wv8)
    Wa_f8 = consts.tile([P, UCH, C], F8)
    nc.sync.dma_start(out=Wa_f8, in_=Wa8)
    bq_sb = consts.tile([P, UCH], F32)
    nc.sync.dma_start(out=bq_sb, in_=bq.rearrange("(uc p) -> p uc", p=P))
    bk_sb = consts.tile([P, UCH], F32)
    nc.sync.dma_start(out=bk_sb, in_=bk.rearrange("(uc p) -> p uc", p=P))
    bc_sb = consts.tile([P, C], F32)
    nc.sync.dma_start(out=bc_sb, in_=bcrep)
    for k in range(TC // 2):
        nc.sync.dma_start(out=x_sb[:, 2 * k:2 * k + 2, :],
                          in_=xr[:, 2 * k:2 * k + 2, :])

    warm_rhs = consts.tile([P, TBLK], BF16)
    nc.vector.memset(warm_rhs, 0.0)
    # PE warmup during the x DMA (N=512: denser activity for HAM)
    with tc.tile_pool(name="warm", bufs=1, space="PSUM") as warm_pool:
        wtile = warm_pool.tile([P, TBLK], F32, name="warmup")
        for i in range(4):
            nc.tensor.matmul(wtile, lhsT=identity, rhs=warm_rhs,
                             start=(i == 0), stop=(i == 3))

    # --- phases 1+2 interleaved per t-block: transposes for a block's
    # four x tiles, then its kT (+ qT for block 0) and v projections, so
    # projection matmuls overlap the x DMA stream ---
    def proj_group(wpool, W_f8, bias_sb, dst, uc, tb, eng_act):
        wps = wpool.tile([P, TBLK], F32, tag="wps", name="wps")
        for i in range(2):
            nc.tensor.matmul(
                wps,
                lhsT=W_f8[:, 2 * i:2 * i + 2, ts(uc, P)],
                rhs=xT_f8[:, 2 * i:2 * i + 2, ds(tb * TBLK, TBLK)],
                start=(i == 0), stop=(i == 1), perf_mode=DR,
            )
        if eng_act:
            nc.scalar.activation(
                out=dst[:, uc, ds(tb * TBLK, TBLK)], in_=wps,
                func=AF.Identity, bias=bias_sb[:, uc:uc + 1], scale=1.0,
            )
        else:
            nc.vector.tensor_scalar(
                out=dst[:, uc, ds(tb * TBLK, TBLK)], in0=wps,
                scalar1=bias_sb[:, uc:uc + 1], scalar2=None,
                op0=mybir.AluOpType.add,
            )

    with tc.tile_pool(name="tpsum", bufs=3, space="PSUM") as tpsum, \
         tc.tile_pool(name="wpsum", bufs=3, space="PSUM") as wpsum, \
         tc.tile_pool(name="vpsum", bufs=2, space="PSUM") as vpsum:
        for tb in range(NTB):
            for tt in range(tb * 4, tb * 4 + 4):
                # transpose via NORMAL matmul against identity (exact for
                # bf16, pipelines fast and keeps the HAM activity
                # monitor warm); reads the host-cast bf16 x directly
                tps = tpsum.tile([P, CCH, P], F32, tag="tps")
                for cc in range(CCH):
                    nc.tensor.matmul(
                        tps[:, cc, :], lhsT=xbf_sb[:, tt, ts(cc, P)],
                        rhs=identity,
                        start=(cc == 0), stop=(cc == CCH - 1),
                    )
                if tt % 2 == 0:
                    nc.vector.tensor_copy(
                        out=xT_f8[:, :, ts(tt, P)], in_=tps
                    )
                else:
                    nc.scalar.copy(out=xT_f8[:, :, ts(tt, P)], in_=tps)
            for uc in range(UCH):
                proj_group(wpsum, Wk_f8, bk_sb, kT_f8, uc, tb, uc == 0)
            if tb == 0:
                for uc in range(UCH):
                    proj_group(wpsum, Wq_f8, bq_sb, qT_f8, uc, 0, uc == 0)
            # v for tiles of this block: two row tiles share one PSUM
            # bank (the second pair opens with start=False so the
            # bank-wide clear of the first start doesn't zero it)
            for h in range(2):
                t0 = tb * 4 + 2 * h
                vps = vpsum.tile([P, 2, U], F32, tag="vps")
                for j in range(2):
                    for i in range(2):
                        nc.tensor.matmul(
                            vps[:, j, :],
                            lhsT=xT_f8[:, 2 * i:2 * i + 2, ts(t0 + j, P)],
                            rhs=Wv_f8[:, 2 * i:2 * i + 2, :],
                            start=(j == 0 and i == 0),
                            stop=(j == 1 and i == 1), perf_mode=DR,
                        )
                if h == 0:
                    nc.scalar.copy(out=v_sb[:, t0:t0 + 2, 0:U], in_=vps)
                else:
                    nc.vector.tensor_copy(
                        out=v_sb[:, t0:t0 + 2, 0:U], in_=vps
                    )

    # --- phase 3: attention ---
    # PSUM: spsum 2x(2 banks) for scores, apsum "acc" 2x(1 bank) holding
    # two 128-row PV accumulators per bank (second half opens with
    # start=False so the bank clear of the first doesn't zero it), and
    # "misc" 2x(1 bank) for a-transposes / output projection / next-qT.
    spsum = tc.alloc_tile_pool(name="spsum", bufs=2, space="PSUM")
    apsum = tc.alloc_tile_pool(name="apsum", bufs=2, space="PSUM")
    p_pool = tc.alloc_tile_pool(name="p_pool", bufs=14)
    abf_pool = tc.alloc_tile_pool(name="abf_pool", bufs=8)
    rcp_pool = tc.alloc_tile_pool(name="rcp_pool", bufs=3)
    y_pool = tc.alloc_tile_pool(name="y_pool", bufs=2)

    def norm_cast(apss, abfs, tsl):
        """rcp of row sum, then a_bf = aps * rcp (normalized), fp32->bf16."""
        aps = apss[tsl]
        rcp = rcp_pool.tile([P, 1], F32, tag="rcp")
        nc.vector.reciprocal(rcp, aps[:, U:U + 1])
        a_bf = abf_pool.tile([P, U], BF16, tag="abf")
        nc.vector.tensor_scalar(
            out=a_bf, in0=aps[:, 0:U], scalar1=rcp, scalar2=None,
            op0=mybir.AluOpType.mult,
        )
        abfs[tsl] = a_bf

    def deferred_work(tb, abfs):
        """Transposes of a (tb), then output projection + residual (tb).
        y tiles pair up for one DMA per two row tiles."""
        chunks = []
        y2box = [None]
        for tsl in range(NTB):
            def tchunk(tsl=tsl, tb=tb, abfs=abfs):
                row0 = tb * TBLK + tsl * P
                atps = apsum.tile([P, UCH, P], F32, tag="misc", name="atps")
                for uc in range(UCH):
                    nc.tensor.matmul(
                        atps[:, uc, :], lhsT=abfs[tsl][:, ts(uc, P)],
                        rhs=identity,
                        start=(uc == 0), stop=(uc == UCH - 1),
                    )
                nc.vector.tensor_copy(out=aT_f8[:, :, ds(row0, P)], in_=atps)
            chunks.append(tchunk)
        for tsl in range(NTB):
            def fchunk(tsl=tsl, tb=tb, y2box=y2box):
                row0 = tb * TBLK + tsl * P
                yps = apsum.tile([P, TBLK], F32, tag="misc", name="yps")
                nc.tensor.matmul(
                    yps, lhsT=aT_f8[:, :, ds(row0, P)], rhs=Wa_f8[:, :, :],
                    start=True, stop=True, perf_mode=DR,
                )
                if tsl % 2 == 0:
                    y2box[0] = y_pool.tile([P, 2, C], F32, tag="ysb",
                                           name="y2")
                y2 = y2box[0]
                nc.vector.tensor_add(
                    out=y2[:, tsl % 2, :], in0=yps,
                    in1=x_sb[:, tb * NTB + tsl, :]
                )
                if tsl % 2 == 1:
                    orow = tb * TBLK + (tsl - 1) * P
                    nc.sync.dma_start(
                        out=out[ds(orow, 2 * P), :].rearrange(
                            "(j p) c -> p j c", p=P),
                        in_=y2,
                    )
            chunks.append(fchunk)
        return chunks

    def emit_scp(tb, scp, pts):
        sps = spsum.tile([P, 2, TBLK], F32, tag="sps", name="sps")
        for j in range(2):
            nc.tensor.matmul(
                sps[:, j, :],
                lhsT=kT_f8[:, :, ts(2 * scp + j, P)],
                rhs=qT_f8[:, :, ds(tb * TBLK, TBLK)],
                start=True, stop=True, perf_mode=DR,
            )
        pt = p_pool.tile([P, 2, TBLK], F8, tag="pt", name="pt")
        nc.scalar.activation(out=pt, in_=sps, func=AF.Exp,
                             bias=expb[:, 0:1], scale=SCALE)
        pts.append(pt)

    deferred = []
    nextpts = []
    for tb in range(NTB):
        pts = nextpts  # scp0 may have been hoisted into tb-1's tail
        nextpts = []
        abfs = [None] * NTB
        apss = [None] * NTB
        for tsl in (0, 1):
            apss[tsl] = apsum.tile([P, VF], F32, tag="acc", name="apsA")
        todo = list(deferred)  # deferred chunks from tb-1
        # qT for block tb+1 joins the deferred queue (just needs xT)
        if tb + 1 < NTB:
            for uc in range(UCH):
                def qchunk(uc=uc, tb=tb):
                    wps = apsum.tile([P, TBLK], F32, tag="misc", name="qps")
                    for i in range(2):
                        nc.tensor.matmul(
                            wps,
                            lhsT=Wq_f8[:, 2 * i:2 * i + 2, ts(uc, P)],
                            rhs=xT_f8[:, 2 * i:2 * i + 2,
                                      ds((tb + 1) * TBLK, TBLK)],
                            start=(i == 0), stop=(i == 1), perf_mode=DR,
                        )
                    nc.scalar.activation(
                        out=qT_f8[:, uc, ds((tb + 1) * TBLK, TBLK)],
                        in_=wps,
                        func=AF.Identity, bias=bq_sb[:, uc:uc + 1],
                        scale=1.0,
                    )
                todo.append(qchunk)
        npv = [0]  # sweep-A pairs emitted so far

        def pva_upto(limit):
            while npv[0] < limit:
                j = npv[0]
                for tsl in (0, 1):
                    nc.tensor.matmul(
                        apss[tsl],
                        lhsT=pts[j][:, :, ts(tsl, P)],
                        rhs=v_sb[:, 2 * j:2 * j + 2, :],
                        start=(j == 0), stop=False, perf_mode=DR,
                    )
                npv[0] += 1

        for scp in range(len(pts), 8):
            emit_scp(tb, scp, pts)
            # PV sweep A (row tiles 0,1), one pair behind the exp
            pva_upto(scp)
            # interleave deferred transposes/output-proj of tb-1
            if scp >= 2:
                while todo and len(todo) > (7 - scp):
                    todo.pop(0)()
            # residual bias add for this block's x tiles (needed by the
            # output projection one block later)
            if scp >= 4:
                tt = tb * 4 + scp - 4
                nc.vector.tensor_add(out=x_sb[:, tt, :],
                                     in0=x_sb[:, tt, :], in1=bc_sb)
        pva_upto(7)
        for tsl in (0, 1):
            nc.tensor.matmul(
                apss[tsl], lhsT=pts[7][:, :, ts(tsl, P)],
                rhs=v_sb[:, 14:16, :], start=False, stop=True, perf_mode=DR,
            )
        while todo:
            todo.pop(0)()
        if tb + 1 < NTB:
            # hoist the next block's first two score groups so their
            # exps run on ScalarE while sweep B occupies the PE
            emit_scp(tb + 1, 0, nextpts)
            emit_scp(tb + 1, 1, nextpts)
        norm_cast(apss, abfs, 0)
        norm_cast(apss, abfs, 1)
        # PV sweep B (row tiles 2,3) over the retained p tiles; on the
        # last block its own transpose/output-projection chunks for row
        # tiles 0,1 interleave into the sweep to shorten the tail
        last = tb == NTB - 1
        if last:
            ch = deferred_work(tb, abfs)
            # reorder: T0,T1,F0,F1 first (only need norms 0,1), then
            # T2,T3,F2,F3 (need norms 2,3 - after the sweep)
            tail = [ch[0], ch[1], ch[4], ch[5], ch[2], ch[3], ch[6], ch[7]]
        else:
            tail = []
        for tsl in (2, 3):
            apss[tsl] = apsum.tile([P, VF], F32, tag="acc", name="apsB")
        for scp in range(8):
            for tsl in (2, 3):
                nc.tensor.matmul(
                    apss[tsl],
                    lhsT=pts[scp][:, :, ts(tsl, P)],
                    rhs=v_sb[:, 2 * scp:2 * scp + 2, :],
                    start=(scp == 0), stop=(scp == 7), perf_mode=DR,
                )
            if last and scp >= 4 and len(tail) > 7 - scp + 4:
                tail.pop(0)()
        norm_cast(apss, abfs, 2)
        norm_cast(apss, abfs, 3)
        if not last:
            deferred = deferred_work(tb, abfs)
        else:
            while tail:
                tail.pop(0)()

    for pool in (y_pool, rcp_pool, abf_pool, p_pool,
                 apsum, spsum, persist, consts):
        pool.release()


def _get_nc():
    if "nc" not in _cache:
        nc = bacc.Bacc("TRN2", target_bir_lowering=False, debug=False)
        with tile.TileContext(nc) as tc:
            _build_kernel(tc)
        nc.compile()
        _cache["nc"] = nc
    return _cache["nc"]


def _w8(w, chunks):
    """fp32 [K, N] -> fp8e4m3 [P, K//P, N] with K-chunk layout for lhsT."""
    f8 = w.reshape(chunks, P, -1).transpose(1, 0, 2)
    return np.ascontiguousarray(f8.astype(ml_dtypes.float8_e4m3))


def _host_inputs(inputs):
    f32 = np.float32
    Wa = np.ascontiguousarray(np.asarray(inputs["Wa"], dtype=f32))
    bc = np.asarray(inputs["bv"], dtype=f32) @ Wa + np.asarray(
        inputs["ba"], dtype=f32
    )
    bcrep = np.ascontiguousarray(
        np.broadcast_to(bc[None, :], (P, C)), dtype=f32
    )
    shared = {
        "Wq8": _w8(np.asarray(inputs["Wq"], dtype=f32), CCH),
        "Wk8": _w8(np.asarray(inputs["Wk"], dtype=f32), CCH),
        "Wv8": _w8(np.asarray(inputs["Wv"], dtype=f32), CCH),
        "Wa8": _w8(Wa, UCH),
        "bq": np.ascontiguousarray(np.asarray(inputs["bq"], dtype=f32)),
        "bk": np.ascontiguousarray(np.asarray(inputs["bk"], dtype=f32)),
        "bcrep": bcrep,
    }
    xs = np.ascontiguousarray(np.asarray(inputs["x"], dtype=f32))
    xbs = np.ascontiguousarray(xs.astype(ml_dtypes.bfloat16))
    return [dict(shared, x=xs[b], xbf=xbs[b]) for b in range(B)]


def kernel(**inputs):
    nc = _get_nc()
    in_maps = _host_inputs(inputs)
    res = run_bass_kernel_spmd(nc, in_maps, core_ids=list(range(B)))
    return np.stack([res.results[b]["out"] for b in range(B)], axis=0)
